# revision 1
# baseline (speedup 1.0000x reference)
"""CapsLayer2D Trainium2 kernel (8-core SPMD, data-parallel over batch).

Math: per position p (of B*R*C) and capsule n:
  U[n,i,o] = sum_e x[p,i,e] * W[n,i,e,o]          (u_hat)
  b0 = 1/64; 2x { v = squash(sum_i b*U); b += sum_o U*v }; out = squash(sum_i b*U)

Mapping:
  - 8 cores, 2 batches each -> 392 positions/core, 4 pos-blocks of 98.
  - Phase 1: S[p,n,o] = sum_{i,e} x*W as dense K=1024 accumulating matmuls
    (v0 = squash(S/64) since b0 is uniform).
  - Phase 2: per (block, n-pair) unit, u_hat materialized into PSUM via
    block-diagonal-W matmuls (stationary = xT chunk, moving = BD(W), N=256),
    then routing iterations as DVE mul + segmented-reduce ops reading PSUM.
  - Host pre-builds xT (transposed inputs), BD(W), dense W.
"""
import numpy as np

import concourse.bacc as bacc
import concourse.bass as bass
import concourse.mybir as mybir
import concourse.tile as tile
from concourse.bass_utils import run_bass_kernel_spmd

N_CORES = 8
B, R, C = 16, 14, 14
N_IN, D_IN = 64, 16          # i, e
N_CAPS, CAPS_DIM = 10, 16    # n, o
IE = N_IN * D_IN             # 1024
POS = (B // N_CORES) * R * C # 392 positions per core
BLK = 98                     # pos-block size
NBLK = POS // BLK            # 4
NF = N_CAPS // 2             # 5 units of 2 capsules
NCH = IE // 128              # 8 contraction chunks
F32 = mybir.dt.float32

# u_hat matmuls run in bf16 (1 col/cycle at any N; fp32 is 4x slower,
# fp32r needs producer-side rounding the DMA can't provide).
BF16 = mybir.dt.bfloat16


def _squash(nc, pool, s_ap, v_ap, n):
    """v = squash(s): s_ap/v_ap are [98, n, 16] APs; n capsules."""
    P = s_ap.shape[0]
    sq = pool.tile([P, n * 16], F32, tag="sq")
    nc.scalar.activation(sq[:].rearrange("p (n o) -> p n o", o=16), s_ap,
                         mybir.ActivationFunctionType.Square)
    q = pool.tile([P, n], F32, tag="q")
    nc.vector.tensor_reduce(q[:], sq[:].rearrange("p (n o) -> p n o", o=16),
                            axis=mybir.AxisListType.X, op=mybir.AluOpType.add)
    rt = pool.tile([P, n], F32, tag="rt")
    nc.scalar.activation(rt[:], q[:], mybir.ActivationFunctionType.Sqrt)
    qp = pool.tile([P, n], F32, tag="qp")
    nc.vector.tensor_scalar_add(qp[:], q[:], 1.0)
    rc = pool.tile([P, n], F32, tag="rc")
    nc.vector.reciprocal(rc[:], qp[:])
    al = pool.tile([P, n], F32, tag="al")
    nc.vector.tensor_mul(al[:], rt[:], rc[:])
    alb = al[:].unsqueeze(2).broadcast_to([P, n, 16])
    nc.vector.tensor_mul(v_ap, s_ap, alb)


def build_kernel(dbg=False, repeat=1):
    nc = bacc.Bacc("TRN2", target_bir_lowering=False, debug=False,
                   num_devices=N_CORES)
    xT = nc.dram_tensor("xT", [IE, POS], F32, kind="ExternalInput").ap()
    bdw = nc.dram_tensor("bdw", [128, NCH * N_CAPS * 128], BF16,
                         kind="ExternalInput").ap()
    wd = nc.dram_tensor("wd", [IE, N_CAPS * 16], F32, kind="ExternalInput").ap()
    out = nc.dram_tensor("out", [POS, N_CAPS * 16], F32,
                         kind="ExternalOutput").ap()
    if dbg:
        dbg_s0 = nc.dram_tensor("dbg_s0", [BLK, NBLK * 160], F32,
                                kind="ExternalOutput").ap()
        dbg_v0 = nc.dram_tensor("dbg_v0", [BLK, NBLK * 160], F32,
                                kind="ExternalOutput").ap()
        dbg_u = nc.dram_tensor("dbg_u", [BLK, 2048], F32,
                               kind="ExternalOutput").ap()
        dbg_b1 = nc.dram_tensor("dbg_b1", [BLK, 128], F32,
                                kind="ExternalOutput").ap()

    with tile.TileContext(nc) as tc:
        for _rep in range(repeat):
            with tc.tile_pool(name="const", bufs=1) as const, \
                 tc.tile_pool(name="work", bufs=3) as work:
                bdw_t = const.tile([128, NCH * N_CAPS * 128], BF16)
                nc.sync.dma_start(bdw_t[:], bdw[:])
                xtb_t = const.tile([128, NCH * POS], BF16)   # bf16 xT for u_hat
                s0_t = const.tile([BLK, NBLK * 160], F32)    # S/64 per block
                v0_t = const.tile([BLK, NBLK * 160], F32)
                out_t = const.tile([BLK, NBLK * 160], F32)

                # ---- phase 1: S = sum_ie x*W ; v0 = squash(S/64) ----
                with tc.tile_pool(name="p1", bufs=1) as p1, \
                     tc.tile_pool(name="psum_s", bufs=4, space="PSUM") as psum_s:
                    xt_t = p1.tile([128, NCH * POS], F32)    # chunk g at g*POS
                    for g in range(NCH):
                        nc.sync.dma_start(xt_t[:, g * POS:(g + 1) * POS],
                                          xT[g * 128:(g + 1) * 128, :])
                    wd_t = p1.tile([128, NCH * N_CAPS * 16], F32)
                    for g in range(NCH):
                        nc.sync.dma_start(wd_t[:, g * 160:(g + 1) * 160],
                                          wd[g * 128:(g + 1) * 128, :])
                    nc.vector.tensor_copy(xtb_t[:], xt_t[:])
                    for b in range(NBLK):
                        for f in range(NF):
                            ps = psum_s.tile([BLK, 32], F32, tag="ps")
                            for g in range(NCH):
                                nc.tensor.matmul(
                                    ps[:],
                                    xt_t[:, g * POS + b * BLK: g * POS + (b + 1) * BLK],
                                    wd_t[:, g * 160 + f * 32: g * 160 + (f + 1) * 32],
                                    start=(g == 0), stop=(g == NCH - 1))
                            nc.scalar.activation(
                                s0_t[:, b * 160 + f * 32: b * 160 + (f + 1) * 32],
                                ps[:], mybir.ActivationFunctionType.Copy,
                                scale=1.0 / N_IN)
                    for b in range(NBLK):
                        sb = s0_t[:, b * 160:(b + 1) * 160].rearrange(
                            "p (n o) -> p n o", o=16)
                        vb = v0_t[:, b * 160:(b + 1) * 160].rearrange(
                            "p (n o) -> p n o", o=16)
                        _squash(nc, work, sb, vb, N_CAPS)

                # ---- phase 2: u_hat + 2 routing iterations, batched per block --
                # Unit (b,f) u_hat -> PSUM [p,(gi,n2,o)] (gi=8g+i8=i), ACT-drains
                # to bf16 SBUF ub[f*2048:]. Routing per block (5 units at once):
                #   it0: b1 = 1/64 + sum_o U*v0 ; v1 = squash(sum_i b1*U)
                #   it1: b2 = b1 + sum_o U*v1  ; out = squash(sum_i b2*U)
                # Products are bf16 DVE muls (2x mode); contractions are halving
                # add-trees (bf16 2x on wide levels, f32 tail) - ~2x faster than
                # tensor_reduce which has no 2x mode.
                with tc.tile_pool(name="ubp", bufs=2) as ubp, \
                     tc.tile_pool(name="big", bufs=1) as big, \
                     tc.tile_pool(name="psum_u", bufs=2, space="PSUM") as psum_u:
                    for b in range(NBLK):
                        ub = ubp.tile([BLK, NF * 2048], BF16, tag="ub")
                        for f in range(NF):
                            up = psum_u.tile([BLK, 2048], F32, tag="up")
                            for g in range(NCH):
                                lhs = xtb_t[:, g * POS + b * BLK: g * POS + (b + 1) * BLK]
                                rhs = bdw_t[:, g * 1280:(g + 1) * 1280] \
                                    .rearrange("p (i c) -> p i c", c=160) \
                                    [:, :, f * 32:(f + 1) * 32]
                                nc.tensor.matmul(
                                    up[:, g * 256:(g + 1) * 256], lhs, rhs,
                                    start=True, stop=True)
                            nc.scalar.activation(ub[:, f * 2048:(f + 1) * 2048],
                                                 up[:],
                                                 mybir.ActivationFunctionType.Copy)
                            if dbg and b == 0 and f == 0:
                                ucp = work.tile([BLK, 2048], F32, tag="ucp")
                                nc.vector.tensor_copy(ucp[:], up[:])
                                nc.sync.dma_start(dbg_u[:], ucp[:])

                        bco = work.tile([BLK, NF * 128], F32, tag="bco")  # (f,gi,n2)
                        nc.vector.memset(bco[:], 1.0 / N_IN)
                        vb16 = work.tile([BLK, 160], BF16, tag="vb16")
                        nc.vector.tensor_copy(vb16[:],
                                              v0_t[:, b * 160:(b + 1) * 160])
                        Ub = ub[:].rearrange("p (f gi no) -> p f gi no",
                                             f=NF, gi=64, no=32)
                        for it in range(2):
                            # agreement: bco += sum_o U*v (tree over o=16)
                            P = big.tile([BLK, NF * 2048], BF16, tag="P")
                            vbb = vb16[:].rearrange("p (f no) -> p f no", no=32) \
                                .unsqueeze(2).broadcast_to([BLK, NF, 64, 32])
                            nc.vector.tensor_mul(
                                P[:].rearrange("p (f gi no) -> p f gi no",
                                               f=NF, gi=64, no=32), Ub, vbb)
                            with nc.allow_low_precision("bf16 tree sums"):
                                Pv = P[:].rearrange("p (s o) -> p s o", o=16)
                                t1 = big.tile([BLK, NF * 1024], BF16, tag="t1")
                                t1v = t1[:].rearrange("p (s o) -> p s o", o=8)
                                nc.vector.tensor_add(t1v, Pv[:, :, 0:8], Pv[:, :, 8:16])
                                t2 = big.tile([BLK, NF * 512], BF16, tag="t2")
                                t2v = t2[:].rearrange("p (s o) -> p s o", o=4)
                                nc.vector.tensor_add(t2v, t1v[:, :, 0:4], t1v[:, :, 4:8])
                                t3 = big.tile([BLK, NF * 256], BF16, tag="t3")
                                t3v = t3[:].rearrange("p (s o) -> p s o", o=2)
                                nc.vector.tensor_add(t3v, t2v[:, :, 0:2], t2v[:, :, 2:4])
                                agr = work.tile([BLK, NF * 128], F32, tag="agr")
                                nc.vector.tensor_add(
                                    agr[:].rearrange("p (s o) -> p s o", o=1),
                                    t3v[:, :, 0:1], t3v[:, :, 1:2])
                            nc.vector.tensor_add(bco[:], bco[:], agr[:])
                            if dbg and b == 0 and it == 0:
                                nc.sync.dma_start(dbg_b1[:], bco[:, 0:128])
                            # v-sum: s = sum_gi b*U (Q in (f,n,o,gi); tree over gi)
                            bcb = work.tile([BLK, NF * 128], BF16, tag="bcb")
                            nc.vector.tensor_copy(bcb[:], bco[:])
                            Q = big.tile([BLK, NF * 2048], BF16, tag="Q")
                            for f in range(NF):
                                Uq = ub[:, f * 2048:(f + 1) * 2048].rearrange(
                                    "p (gi n o) -> p n o gi", gi=64, n=2, o=16)
                                bbf = bcb[:, f * 128:(f + 1) * 128].rearrange(
                                    "p (gi n) -> p n gi", n=2) \
                                    .unsqueeze(2).broadcast_to([BLK, 2, 16, 64])
                                nc.vector.tensor_mul(
                                    Q[:, f * 2048:(f + 1) * 2048].rearrange(
                                        "p (n o gi) -> p n o gi", n=2, o=16),
                                    Uq, bbf)
                            with nc.allow_low_precision("bf16 tree sums"):
                                Qv = Q[:].rearrange("p (s g) -> p s g", g=64)
                                q1 = big.tile([BLK, NF * 1024], BF16, tag="q1")
                                q1v = q1[:].rearrange("p (s g) -> p s g", g=32)
                                nc.vector.tensor_add(q1v, Qv[:, :, 0:32], Qv[:, :, 32:64])
                                q2 = big.tile([BLK, NF * 512], BF16, tag="q2")
                                q2v = q2[:].rearrange("p (s g) -> p s g", g=16)
                                nc.vector.tensor_add(q2v, q1v[:, :, 0:16], q1v[:, :, 16:32])
                                q3 = big.tile([BLK, NF * 256], BF16, tag="q3")
                                q3v = q3[:].rearrange("p (s g) -> p s g", g=8)
                                nc.vector.tensor_add(q3v, q2v[:, :, 0:8], q2v[:, :, 8:16])
                                q4 = big.tile([BLK, NF * 128], BF16, tag="q4")
                                q4v = q4[:].rearrange("p (s g) -> p s g", g=4)
                                nc.vector.tensor_add(q4v, q3v[:, :, 0:4], q3v[:, :, 4:8])
                                q5 = big.tile([BLK, NF * 64], BF16, tag="q5")
                                q5v = q5[:].rearrange("p (s g) -> p s g", g=2)
                                nc.vector.tensor_add(q5v, q4v[:, :, 0:2], q4v[:, :, 2:4])
                                s_blk = work.tile([BLK, 160], F32, tag="s_blk")
                                nc.vector.tensor_add(
                                    s_blk[:].rearrange("p (s g) -> p s g", g=1),
                                    q5v[:, :, 0:1], q5v[:, :, 1:2])
                            # squash (batched over the block's 10 capsules)
                            if it == 0:
                                v_blk = work.tile([BLK, 160], F32, tag="v_blk")
                                _squash(nc, work,
                                        s_blk[:].rearrange("p (n o) -> p n o", o=16),
                                        v_blk[:].rearrange("p (n o) -> p n o", o=16),
                                        N_CAPS)
                                nc.vector.tensor_copy(vb16[:], v_blk[:])
                            else:
                                dst = out_t[:, b * 160:(b + 1) * 160]
                                _squash(nc, work,
                                        s_blk[:].rearrange("p (n o) -> p n o", o=16),
                                        dst.rearrange("p (n o) -> p n o", o=16),
                                        N_CAPS)

                for b in range(NBLK):
                    nc.sync.dma_start(out[b * BLK:(b + 1) * BLK, :],
                                      out_t[:, b * 160:(b + 1) * 160])
                if dbg:
                    nc.sync.dma_start(dbg_s0[:], s0_t[:])
                    nc.sync.dma_start(dbg_v0[:], v0_t[:])
    nc.compile()
    return nc


def _host_prep(inputs, W):
    """Build per-core input maps from full inputs."""
    x = np.ascontiguousarray(inputs, dtype=np.float32).reshape(B, R * C, IE)
    Wf = np.ascontiguousarray(W, dtype=np.float32)  # [n, i, e, o]
    # bdw[(i8,e), (g,n,i8,o)]
    Wg = Wf.reshape(N_CAPS, 8, 8, D_IN, CAPS_DIM)   # [n, g, i8, e, o]
    bdw6 = np.zeros((8, D_IN, 8, 8, N_CAPS, CAPS_DIM), dtype=np.float32)
    for i8 in range(8):
        # [n, g, e, o] -> [e, g, n, o]
        bdw6[i8, :, :, i8, :, :] = Wg[:, :, i8, :, :].transpose(2, 1, 0, 3)
    import ml_dtypes
    bdw = bdw6.reshape(128, NCH * N_CAPS * 128).astype(ml_dtypes.bfloat16)
    wd = Wf.transpose(1, 2, 0, 3).reshape(IE, N_CAPS * CAPS_DIM)
    bpc = B // N_CORES
    in_maps = []
    for c in range(N_CORES):
        xc = x[c * bpc:(c + 1) * bpc].reshape(POS, IE)
        in_maps.append({
            "xT": np.ascontiguousarray(xc.T),
            "bdw": bdw,
            "wd": wd,
        })
    return in_maps


_NC_CACHE = []


def kernel(inputs: np.ndarray, W: np.ndarray) -> np.ndarray:
    in_maps = _host_prep(inputs, W)
    if not _NC_CACHE:
        _NC_CACHE.append(build_kernel())
    nc = _NC_CACHE[0]
    res = run_bass_kernel_spmd(nc, in_maps, list(range(N_CORES)))
    outs = [res.results[c]["out"] for c in range(N_CORES)]
    full = np.concatenate(outs, axis=0)  # [3136, 160]
    return full.reshape(B, R, C, N_CAPS, CAPS_DIM)



# revision 13
# speedup vs baseline: 3.7265x; 3.7265x over previous
"""CapsLayer2D Trainium2 kernel (8-core SPMD, data-parallel over batch).

Math per position p (of B*R*C) and capsule n:
  U[n,i,o] = sum_e x[p,i,e] * W[n,i,e,o]          (u_hat)
  b0 = 1/64; 2x { v = squash(sum_i b*U); b += sum_o U*v }; out = squash(sum_i b*U)

Routing is algebraically restated without the b-state:
  s_mean = (1/64) sum_i U_i ; v0 = squash(s_mean); s(0) = s_mean
  iter t: a_i = U_i . v_t ; s(t+1) = s(t) + sum_i a_i U_i ; v_{t+1} = squash(s(t+1))
(identical results: b_t = 1/64 + U.(v0+..+v_{t-1}) telescopes into s).

Mapping:
  - 8 cores, 2 batches each -> 392 positions/core, 4 pos-blocks of 98.
  - Per block: s_mean via one dense bf16 matmul (K=1024, N=160); u_hat via
    block-diagonal-W bf16 matmuls (PSUM cols (gi,o,n2)), ACT-drained into
    U[p, (gi, o, n)] bf16; then 2 routing iterations on DVE.
    All routing DVE ops run in 2x mode: broadcasts sit on non-innermost dims,
    tree-sum halves stay contiguous (o middle -> o-tree 2x; gi outermost ->
    i-tree halves are contiguous monoliths).
  - Output v-layout is (o, n) per position; host transposes to (n, o).
"""
import numpy as np

import concourse.bacc as bacc
import concourse.bass as bass
import concourse.mybir as mybir
import concourse.tile as tile
from concourse.bass_utils import run_bass_kernel_spmd

N_CORES = 8
B, R, C = 16, 14, 14
N_IN, D_IN = 64, 16          # i, e
N_CAPS, CAPS_DIM = 10, 16    # n, o
IE = N_IN * D_IN             # 1024
POS = (B // N_CORES) * R * C # 392 positions per core
BLK = 98                     # pos-block size
NBLK = POS // BLK            # 4
NF = N_CAPS // 2             # 5 units of 2 capsules
NCH = IE // 128              # 8 contraction chunks
F32 = mybir.dt.float32
BF16 = mybir.dt.bfloat16
AF = mybir.ActivationFunctionType


def _squash_on(nc, pool, s_ap, v_ap):
    """v = squash(s) in (o, n) free layout. s_ap [P,160] f32, v_ap [P,160].

    Square and the Sqrt-independent steps run on DVE before the single ACT
    Sqrt dependency, minimizing the DVE stall on the ACT round-trip."""
    P = s_ap.shape[0]
    sq = pool.tile([P, 160], F32, tag="sq")
    nc.vector.tensor_mul(sq[:], s_ap, s_ap)
    q = pool.tile([P, N_CAPS], F32, tag="q")
    nc.vector.tensor_reduce(q[:], sq[:].rearrange("p (o n) -> p n o", n=N_CAPS),
                            axis=mybir.AxisListType.X, op=mybir.AluOpType.add)
    qp = pool.tile([P, N_CAPS], F32, tag="qp")
    nc.vector.tensor_scalar_add(qp[:], q[:], 1.0)
    rc = pool.tile([P, N_CAPS], F32, tag="rc")
    nc.vector.reciprocal(rc[:], qp[:])
    rt = pool.tile([P, N_CAPS], F32, tag="rt")
    nc.scalar.activation(rt[:], q[:], AF.Sqrt)
    al = pool.tile([P, N_CAPS], F32, tag="al")
    nc.vector.tensor_mul(al[:], rt[:], rc[:])
    alb = al[:].unsqueeze(1).broadcast_to([P, CAPS_DIM, N_CAPS])
    nc.vector.tensor_mul(v_ap.rearrange("p (o n) -> p o n", n=N_CAPS),
                         s_ap.rearrange("p (o n) -> p o n", n=N_CAPS), alb)


def build_kernel(dbg=False, repeat=1):
    nc = bacc.Bacc("TRN2", target_bir_lowering=False, debug=False,
                   num_devices=N_CORES)
    xTb = nc.dram_tensor("xTb", [IE, POS], BF16, kind="ExternalInput").ap()
    # bdw: [128=(i8,e), (g,f) * 256=(i8,o,n2)] block-diag W, bf16
    bdw = nc.dram_tensor("bdw", [128, NCH * NF * 256], BF16,
                         kind="ExternalInput").ap()
    # wdb: [IE, 160=(o,n)] dense W for s_mean
    wdb = nc.dram_tensor("wdb", [IE, N_CAPS * 16], BF16,
                         kind="ExternalInput").ap()
    # out rows = positions, cols = (o, n)
    out = nc.dram_tensor("out", [POS, N_CAPS * 16], F32,
                         kind="ExternalOutput").ap()

    with tile.TileContext(nc) as tc:
        for _rep in range(repeat):
            with tc.tile_pool(name="const", bufs=1) as const, \
                 tc.tile_pool(name="work", bufs=2) as work, \
                 tc.tile_pool(name="ubp", bufs=2) as ubp, \
                 tc.tile_pool(name="big", bufs=1) as big, \
                 tc.tile_pool(name="psum_u", bufs=2, space="PSUM") as psum_u:
                # Warm the ACT function tables (Copy/Sqrt) during the input
                # DMAs instead of paying the ~1.3us load on the critical path.
                warm = const.tile([1, 2], F32)
                nc.vector.memset(warm[:], 1.0)
                nc.scalar.activation(warm[:, 0:1], warm[:, 1:2], AF.Copy)
                nc.scalar.activation(warm[:, 0:1], warm[:, 1:2], AF.Sqrt)
                # xtb + wd first: they gate s_mean(0); bdw only gates u_hat(0).
                # Spread issue across engine queues so HWDGE issue pipelines.
                dmae = [nc.sync, nc.scalar, nc.sync, nc.scalar]
                xtb_t = const.tile([128, NCH * POS], BF16)
                for g in range(NCH):
                    dmae[g % 4].dma_start(xtb_t[:, g * POS:(g + 1) * POS],
                                          xTb[g * 128:(g + 1) * 128, :])
                wd_t = const.tile([128, NCH * 160], BF16)
                for g in range(NCH):
                    dmae[g % 2].dma_start(wd_t[:, g * 160:(g + 1) * 160],
                                          wdb[g * 128:(g + 1) * 128, :])
                bdw_t = const.tile([128, NCH * NF * 256], BF16)
                for g in range(NCH):   # per-chunk DMAs parallelize the queues
                    dmae[g % 4].dma_start(
                        bdw_t[:, g * NF * 256:(g + 1) * NF * 256],
                        bdw[:, g * NF * 256:(g + 1) * NF * 256])
                sacc = const.tile([BLK, NBLK * 160], F32)   # s per block, (o,n)
                v0_t = const.tile([BLK, NBLK * 160], BF16)

                # ---- prologue: s_mean(b) for all blocks; v0 = squash ----
                # (keeps these short ACT/DVE chains off the routing's critical
                # path -- the U-drains otherwise queue ahead of them on ACT)
                for b in range(NBLK):
                    sb = sacc[:, b * 160:(b + 1) * 160]
                    ps = psum_u.tile([BLK, 160], F32, tag="ps")
                    for g in range(NCH):
                        nc.tensor.matmul(
                            ps[:],
                            xtb_t[:, g * POS + b * BLK: g * POS + (b + 1) * BLK],
                            wd_t[:, g * 160:(g + 1) * 160],
                            start=(g == 0), stop=(g == NCH - 1))
                    nc.scalar.activation(sb, ps[:], AF.Copy, scale=1.0 / N_IN)
                    _squash_on(nc, work, sb, v0_t[:, b * 160:(b + 1) * 160])

                for b in range(NBLK):
                    sb = sacc[:, b * 160:(b + 1) * 160]
                    v0 = v0_t[:, b * 160:(b + 1) * 160]
                    # ---- u_hat(b) -> U[p, (gi, o, n)] bf16 ----
                    U = ubp.tile([BLK, 10240], BF16, tag="U")
                    Uv = U[:].rearrange("p (gi o n) -> p gi o n",
                                        gi=64, o=16, n=N_CAPS)
                    for f in range(NF):
                        for h in range(2):   # chunk halves: g in [4h, 4h+4)
                            up = psum_u.tile([BLK, 1024], F32, tag="up")
                            for gg in range(4):
                                g = 4 * h + gg
                                nc.tensor.matmul(
                                    up[:, gg * 256:(gg + 1) * 256],
                                    xtb_t[:, g * POS + b * BLK: g * POS + (b + 1) * BLK],
                                    bdw_t[:, (g * NF + f) * 256:(g * NF + f + 1) * 256],
                                    start=True, stop=True)
                            # PSUM cols (i8,o,n2) per chunk -> merged (gi,o,n2)
                            nc.scalar.activation(
                                Uv[:, 32 * h:32 * (h + 1), :, 2 * f:2 * f + 2],
                                up[:].rearrange("p (gi o n) -> p gi o n",
                                                gi=32, o=16, n=2),
                                AF.Copy)

                    # ---- 2 routing iterations ----
                    for it in range(2):
                        v_ap = v0 if it == 0 else v1[:]
                        # P = U * v (bcast over gi: middle dims stay 2x)
                        P = big.tile([BLK, 10240], BF16, tag="P")
                        Pv4 = P[:].rearrange("p (gi o n) -> p gi o n",
                                             gi=64, o=16, n=N_CAPS)
                        if b == 0 and it == 0:
                            # pipeline fill: split per (f,h)-slice so the mul
                            # starts as soon as each U-drain lands
                            for f in range(NF):
                                for h in range(2):
                                    sl = (slice(None), slice(32 * h, 32 * h + 32),
                                          slice(None), slice(2 * f, 2 * f + 2))
                                    vbs = v_ap.rearrange("p (o n) -> p o n",
                                                         n=N_CAPS) \
                                        [:, :, 2 * f:2 * f + 2].unsqueeze(1) \
                                        .broadcast_to([BLK, 32, 16, 2])
                                    nc.vector.tensor_mul(Pv4[sl], Uv[sl], vbs)
                        else:
                            vb = v_ap.rearrange("p (o n) -> p o n", n=N_CAPS) \
                                .unsqueeze(1).broadcast_to([BLK, 64, 16, N_CAPS])
                            nc.vector.tensor_mul(Pv4, Uv, vb)
                        # o-tree (middle-dim halves, contiguous runs)
                        with nc.allow_low_precision("bf16 tree sums"):
                            Pv = P[:].rearrange("p (gi o n) -> p gi o n",
                                                gi=64, o=16, n=N_CAPS)
                            t1 = big.tile([BLK, 5120], BF16, tag="t1")
                            t1v = t1[:].rearrange("p (gi o n) -> p gi o n",
                                                  gi=64, o=8, n=N_CAPS)
                            nc.vector.tensor_add(t1v, Pv[:, :, 0:8, :],
                                                 Pv[:, :, 8:16, :])
                            t2 = big.tile([BLK, 2560], BF16, tag="t2")
                            t2v = t2[:].rearrange("p (gi o n) -> p gi o n",
                                                  gi=64, o=4, n=N_CAPS)
                            nc.vector.tensor_add(t2v, t1v[:, :, 0:4, :],
                                                 t1v[:, :, 4:8, :])
                            t3 = big.tile([BLK, 1280], BF16, tag="t3")
                            t3v = t3[:].rearrange("p (gi o n) -> p gi o n",
                                                  gi=64, o=2, n=N_CAPS)
                            nc.vector.tensor_add(t3v, t2v[:, :, 0:2, :],
                                                 t2v[:, :, 2:4, :])
                            a = big.tile([BLK, 640], BF16, tag="a")
                            av = a[:].rearrange("p (gi o n) -> p gi o n",
                                                gi=64, o=1, n=N_CAPS)
                            nc.vector.tensor_add(av, t3v[:, :, 0:1, :],
                                                 t3v[:, :, 1:2, :])
                        # Q = U * a (bcast over o: middle dim, still 2x)
                        Q = big.tile([BLK, 10240], BF16, tag="Q")
                        ab = a[:].rearrange("p (gi n) -> p gi n", n=N_CAPS) \
                            .unsqueeze(2).broadcast_to([BLK, 64, 16, N_CAPS])
                        nc.vector.tensor_mul(
                            Q[:].rearrange("p (gi o n) -> p gi o n",
                                           gi=64, o=16, n=N_CAPS), Uv, ab)
                        # i-tree (outermost gi halves: contiguous monoliths)
                        with nc.allow_low_precision("bf16 tree sums"):
                            q1 = big.tile([BLK, 5120], BF16, tag="q1")
                            nc.vector.tensor_add(q1[:], Q[:, 0:5120],
                                                 Q[:, 5120:10240])
                            q2 = big.tile([BLK, 2560], BF16, tag="q2")
                            nc.vector.tensor_add(q2[:], q1[:, 0:2560],
                                                 q1[:, 2560:5120])
                            q3 = big.tile([BLK, 1280], BF16, tag="q3")
                            nc.vector.tensor_add(q3[:], q2[:, 0:1280],
                                                 q2[:, 1280:2560])
                            q4 = big.tile([BLK, 640], BF16, tag="q4")
                            nc.vector.tensor_add(q4[:], q3[:, 0:640],
                                                 q3[:, 640:1280])
                            q5 = big.tile([BLK, 320], BF16, tag="q5")
                            nc.vector.tensor_add(q5[:], q4[:, 0:320],
                                                 q4[:, 320:640])
                            inc = work.tile([BLK, 160], F32, tag="inc")
                            nc.vector.tensor_add(inc[:], q5[:, 0:160],
                                                 q5[:, 160:320])
                        nc.vector.tensor_add(sb, sb, inc[:])
                        if it == 0:
                            v1 = work.tile([BLK, 160], BF16, tag="v1")
                            _squash_on(nc, work, sb, v1[:])
                        else:
                            out_t = work.tile([BLK, 160], F32, tag="out_t")
                            _squash_on(nc, work, sb, out_t[:])
                            nc.sync.dma_start(
                                out[b * BLK:(b + 1) * BLK, :], out_t[:])
    nc.compile()
    return nc


def _host_prep(inputs, W):
    """Build per-core input maps from full inputs."""
    import ml_dtypes
    x = np.ascontiguousarray(inputs, dtype=np.float32).reshape(B, R * C, IE)
    Wf = np.ascontiguousarray(W, dtype=np.float32)  # [n, i, e, o]
    # bdw[(i8_r,e), (g, f, i8, o, n2)]: delta(i8_r,i8) * W[2f+n2, 8g+i8, e, o]
    Wg = Wf.reshape(NF, 2, NCH, 8, D_IN, CAPS_DIM)  # [f, n2, g, i8, e, o]
    bdw7 = np.zeros((8, D_IN, NCH, NF, 8, CAPS_DIM, 2), dtype=np.float32)
    for i8 in range(8):
        # [f, n2, g, e, o] -> [e, g, f, o, n2]
        bdw7[i8, :, :, :, i8, :, :] = Wg[:, :, :, i8, :, :].transpose(3, 2, 0, 4, 1)
    bdw = bdw7.reshape(128, NCH * NF * 256).astype(ml_dtypes.bfloat16)
    # wdb[(i,e), (o,n)]
    wdb = np.ascontiguousarray(
        Wf.transpose(1, 2, 3, 0).reshape(IE, CAPS_DIM * N_CAPS)
    ).astype(ml_dtypes.bfloat16)
    bpc = B // N_CORES
    in_maps = []
    for c in range(N_CORES):
        xc = x[c * bpc:(c + 1) * bpc].reshape(POS, IE)
        in_maps.append({
            "xTb": np.ascontiguousarray(xc.T).astype(ml_dtypes.bfloat16),
            "bdw": bdw,
            "wdb": wdb,
        })
    return in_maps


_NC_CACHE = []


def kernel(inputs: np.ndarray, W: np.ndarray) -> np.ndarray:
    in_maps = _host_prep(inputs, W)
    if not _NC_CACHE:
        _NC_CACHE.append(build_kernel())
    nc = _NC_CACHE[0]
    res = run_bass_kernel_spmd(nc, in_maps, list(range(N_CORES)))
    outs = [res.results[c]["out"] for c in range(N_CORES)]
    full = np.concatenate(outs, axis=0)  # [3136, (o,n)]
    return np.ascontiguousarray(
        full.reshape(B, R, C, CAPS_DIM, N_CAPS).transpose(0, 1, 2, 4, 3))


# revision 14
# speedup vs baseline: 4.0400x; 1.0841x over previous
"""CapsLayer2D Trainium2 kernel (8-core SPMD, data-parallel over batch).

Math per position p (of B*R*C) and capsule n:
  U[n,i,o] = sum_e x[p,i,e] * W[n,i,e,o]          (u_hat)
  b0 = 1/64; 2x { v = squash(sum_i b*U); b += sum_o U*v }; out = squash(sum_i b*U)

Routing is algebraically restated without the b-state:
  s_mean = (1/64) sum_i U_i ; v0 = squash(s_mean); s(0) = s_mean
  iter t: a_i = U_i . v_t ; s(t+1) = s(t) + sum_i a_i U_i ; v_{t+1} = squash(s(t+1))
(identical results: b_t = 1/64 + U.(v0+..+v_{t-1}) telescopes into s).

Mapping:
  - 8 cores, 2 batches each -> 392 positions/core, 4 pos-blocks of 98.
  - Per block: s_mean via one dense bf16 matmul (K=1024, N=160); u_hat via
    block-diagonal-W bf16 matmuls (PSUM cols (gi,o,n2)), ACT-drained into
    U[p, (gi, o, n)] bf16; then 2 routing iterations on DVE.
    All routing DVE ops run in 2x mode: broadcasts sit on non-innermost dims,
    tree-sum halves stay contiguous (o middle -> o-tree 2x; gi outermost ->
    i-tree halves are contiguous monoliths).
  - Output v-layout is (o, n) per position; host transposes to (n, o).
"""
import numpy as np

import concourse.bacc as bacc
import concourse.bass as bass
import concourse.mybir as mybir
import concourse.tile as tile
from concourse.bass_utils import run_bass_kernel_spmd

N_CORES = 8
B, R, C = 16, 14, 14
N_IN, D_IN = 64, 16          # i, e
N_CAPS, CAPS_DIM = 10, 16    # n, o
IE = N_IN * D_IN             # 1024
POS = (B // N_CORES) * R * C # 392 positions per core
BLK = 98                     # pos-block size
NBLK = POS // BLK            # 4
NF = N_CAPS // 2             # 5 units of 2 capsules
NCH = IE // 128              # 8 contraction chunks
F32 = mybir.dt.float32
BF16 = mybir.dt.bfloat16
AF = mybir.ActivationFunctionType


def _squash_on(nc, pool, s_ap, v_ap):
    """v = squash(s) in (o, n) free layout. s_ap [P,160] f32, v_ap [P,160].

    Square and the Sqrt-independent steps run on DVE before the single ACT
    Sqrt dependency, minimizing the DVE stall on the ACT round-trip."""
    P = s_ap.shape[0]
    sq = pool.tile([P, 160], F32, tag="sq")
    nc.vector.tensor_mul(sq[:], s_ap, s_ap)
    q = pool.tile([P, N_CAPS], F32, tag="q")
    nc.vector.tensor_reduce(q[:], sq[:].rearrange("p (o n) -> p n o", n=N_CAPS),
                            axis=mybir.AxisListType.X, op=mybir.AluOpType.add)
    qp = pool.tile([P, N_CAPS], F32, tag="qp")
    nc.vector.tensor_scalar_add(qp[:], q[:], 1.0)
    rc = pool.tile([P, N_CAPS], F32, tag="rc")
    nc.vector.reciprocal(rc[:], qp[:])
    rt = pool.tile([P, N_CAPS], F32, tag="rt")
    nc.scalar.activation(rt[:], q[:], AF.Sqrt)
    al = pool.tile([P, N_CAPS], F32, tag="al")
    nc.vector.tensor_mul(al[:], rt[:], rc[:])
    alb = al[:].unsqueeze(1).broadcast_to([P, CAPS_DIM, N_CAPS])
    nc.vector.tensor_mul(v_ap.rearrange("p (o n) -> p o n", n=N_CAPS),
                         s_ap.rearrange("p (o n) -> p o n", n=N_CAPS), alb)


def build_kernel(dbg=False, repeat=1):
    nc = bacc.Bacc("TRN2", target_bir_lowering=False, debug=False,
                   num_devices=N_CORES)
    xTb = nc.dram_tensor("xTb", [IE, POS], BF16, kind="ExternalInput").ap()
    # bdw: [128=(i8,e), (g,f) * 256=(i8,o,n2)] block-diag W, bf16
    bdw = nc.dram_tensor("bdw", [128, NCH * NF * 256], BF16,
                         kind="ExternalInput").ap()
    # wdb: [IE, 160=(o,n)] dense W for s_mean
    wdb = nc.dram_tensor("wdb", [IE, N_CAPS * 16], BF16,
                         kind="ExternalInput").ap()
    # out rows = positions, cols = (o, n)
    out = nc.dram_tensor("out", [POS, N_CAPS * 16], F32,
                         kind="ExternalOutput").ap()

    with tile.TileContext(nc) as tc:
        for _rep in range(repeat):
            with tc.tile_pool(name="const", bufs=1) as const, \
                 tc.tile_pool(name="work", bufs=2) as work, \
                 tc.tile_pool(name="ubp", bufs=2) as ubp, \
                 tc.tile_pool(name="big", bufs=1) as big, \
                 tc.tile_pool(name="psum_u", bufs=2, space="PSUM") as psum_u:
                # Warm the ACT function tables (Copy/Sqrt) during the input
                # DMAs instead of paying the ~1.3us load on the critical path.
                warm = const.tile([1, 2], F32)
                nc.vector.memset(warm[:], 1.0)
                nc.scalar.activation(warm[:, 0:1], warm[:, 1:2], AF.Copy)
                nc.scalar.activation(warm[:, 0:1], warm[:, 1:2], AF.Sqrt)
                # xtb + wd first: they gate s_mean(0); bdw only gates u_hat(0).
                # Spread issue across engine queues so HWDGE issue pipelines.
                dmae = [nc.sync, nc.scalar, nc.sync, nc.scalar]
                xtb_t = const.tile([128, NCH * POS], BF16)
                wd_t = const.tile([128, NCH * 160], BF16)
                bdw_t = const.tile([128, NCH * NF * 256], BF16)
                for g in range(NCH):   # interleave so u_hat(0) streams early
                    nc.sync.dma_start(xtb_t[:, g * POS:(g + 1) * POS],
                                      xTb[g * 128:(g + 1) * 128, :])
                    nc.scalar.dma_start(
                        bdw_t[:, g * NF * 256:(g + 1) * NF * 256],
                        bdw[:, g * NF * 256:(g + 1) * NF * 256])
                for g in range(NCH):
                    dmae[g % 2].dma_start(wd_t[:, g * 160:(g + 1) * 160],
                                          wdb[g * 128:(g + 1) * 128, :])
                sacc = const.tile([BLK, NBLK * 160], F32)   # s per block, (o,n)
                v0_t = const.tile([BLK, NBLK * 160], BF16)

                # ---- prologue: s_mean(b) for all blocks; v0 = squash ----
                # (keeps these short ACT/DVE chains off the routing's critical
                # path -- the U-drains otherwise queue ahead of them on ACT)
                for b in range(NBLK):
                    sb = sacc[:, b * 160:(b + 1) * 160]
                    ps = psum_u.tile([BLK, 160], F32, tag="ps")
                    for g in range(NCH):
                        nc.tensor.matmul(
                            ps[:],
                            xtb_t[:, g * POS + b * BLK: g * POS + (b + 1) * BLK],
                            wd_t[:, g * 160:(g + 1) * 160],
                            start=(g == 0), stop=(g == NCH - 1))
                    nc.scalar.activation(sb, ps[:], AF.Copy, scale=1.0 / N_IN)
                    _squash_on(nc, work, sb, v0_t[:, b * 160:(b + 1) * 160])

                for b in range(NBLK):
                    sb = sacc[:, b * 160:(b + 1) * 160]
                    v0 = v0_t[:, b * 160:(b + 1) * 160]
                    # ---- u_hat(b) -> U[p, (gi, o, n)] bf16 ----
                    U = ubp.tile([BLK, 10240], BF16, tag="U")
                    Uv = U[:].rearrange("p (gi o n) -> p gi o n",
                                        gi=64, o=16, n=N_CAPS)
                    for f in range(NF):
                        for h in range(2):   # chunk halves: g in [4h, 4h+4)
                            up = psum_u.tile([BLK, 1024], F32, tag="up")
                            for gg in range(4):
                                g = 4 * h + gg
                                nc.tensor.matmul(
                                    up[:, gg * 256:(gg + 1) * 256],
                                    xtb_t[:, g * POS + b * BLK: g * POS + (b + 1) * BLK],
                                    bdw_t[:, (g * NF + f) * 256:(g * NF + f + 1) * 256],
                                    start=True, stop=True)
                            # PSUM cols (i8,o,n2) per chunk -> merged (gi,o,n2)
                            nc.scalar.activation(
                                Uv[:, 32 * h:32 * (h + 1), :, 2 * f:2 * f + 2],
                                up[:].rearrange("p (gi o n) -> p gi o n",
                                                gi=32, o=16, n=2),
                                AF.Copy)

                    # ---- 2 routing iterations ----
                    for it in range(2):
                        v_ap = v0 if it == 0 else v1[:]
                        # P = U * v (bcast over gi: middle dims stay 2x)
                        P = big.tile([BLK, 10240], BF16, tag="P")
                        Pv4 = P[:].rearrange("p (gi o n) -> p gi o n",
                                             gi=64, o=16, n=N_CAPS)
                        if b == 0 and it == 0:
                            # pipeline fill: split per (f,h)-slice so the mul
                            # starts as soon as each U-drain lands
                            for f in range(NF):
                                for h in range(2):
                                    sl = (slice(None), slice(32 * h, 32 * h + 32),
                                          slice(None), slice(2 * f, 2 * f + 2))
                                    vbs = v_ap.rearrange("p (o n) -> p o n",
                                                         n=N_CAPS) \
                                        [:, :, 2 * f:2 * f + 2].unsqueeze(1) \
                                        .broadcast_to([BLK, 32, 16, 2])
                                    nc.vector.tensor_mul(Pv4[sl], Uv[sl], vbs)
                        else:
                            vb = v_ap.rearrange("p (o n) -> p o n", n=N_CAPS) \
                                .unsqueeze(1).broadcast_to([BLK, 64, 16, N_CAPS])
                            nc.vector.tensor_mul(Pv4, Uv, vb)
                        # o-tree (middle-dim halves, contiguous runs)
                        with nc.allow_low_precision("bf16 tree sums"):
                            Pv = P[:].rearrange("p (gi o n) -> p gi o n",
                                                gi=64, o=16, n=N_CAPS)
                            t1 = big.tile([BLK, 5120], BF16, tag="t1")
                            t1v = t1[:].rearrange("p (gi o n) -> p gi o n",
                                                  gi=64, o=8, n=N_CAPS)
                            nc.vector.tensor_add(t1v, Pv[:, :, 0:8, :],
                                                 Pv[:, :, 8:16, :])
                            t2 = big.tile([BLK, 2560], BF16, tag="t2")
                            t2v = t2[:].rearrange("p (gi o n) -> p gi o n",
                                                  gi=64, o=4, n=N_CAPS)
                            nc.vector.tensor_add(t2v, t1v[:, :, 0:4, :],
                                                 t1v[:, :, 4:8, :])
                            t3 = big.tile([BLK, 1280], BF16, tag="t3")
                            t3v = t3[:].rearrange("p (gi o n) -> p gi o n",
                                                  gi=64, o=2, n=N_CAPS)
                            nc.vector.tensor_add(t3v, t2v[:, :, 0:2, :],
                                                 t2v[:, :, 2:4, :])
                            a = big.tile([BLK, 640], BF16, tag="a")
                            av = a[:].rearrange("p (gi o n) -> p gi o n",
                                                gi=64, o=1, n=N_CAPS)
                            nc.vector.tensor_add(av, t3v[:, :, 0:1, :],
                                                 t3v[:, :, 1:2, :])
                        # Q = U * a (bcast over o: middle dim, still 2x)
                        Q = big.tile([BLK, 10240], BF16, tag="Q")
                        ab = a[:].rearrange("p (gi n) -> p gi n", n=N_CAPS) \
                            .unsqueeze(2).broadcast_to([BLK, 64, 16, N_CAPS])
                        nc.vector.tensor_mul(
                            Q[:].rearrange("p (gi o n) -> p gi o n",
                                           gi=64, o=16, n=N_CAPS), Uv, ab)
                        # i-tree (outermost gi halves: contiguous monoliths)
                        with nc.allow_low_precision("bf16 tree sums"):
                            q1 = big.tile([BLK, 5120], BF16, tag="q1")
                            nc.vector.tensor_add(q1[:], Q[:, 0:5120],
                                                 Q[:, 5120:10240])
                            q2 = big.tile([BLK, 2560], BF16, tag="q2")
                            nc.vector.tensor_add(q2[:], q1[:, 0:2560],
                                                 q1[:, 2560:5120])
                            q3 = big.tile([BLK, 1280], BF16, tag="q3")
                            nc.vector.tensor_add(q3[:], q2[:, 0:1280],
                                                 q2[:, 1280:2560])
                            q4 = big.tile([BLK, 640], BF16, tag="q4")
                            nc.vector.tensor_add(q4[:], q3[:, 0:640],
                                                 q3[:, 640:1280])
                            q5 = big.tile([BLK, 320], BF16, tag="q5")
                            nc.vector.tensor_add(q5[:], q4[:, 0:320],
                                                 q4[:, 320:640])
                            inc = work.tile([BLK, 160], F32, tag="inc")
                            nc.vector.tensor_add(inc[:], q5[:, 0:160],
                                                 q5[:, 160:320])
                        nc.vector.tensor_add(sb, sb, inc[:])
                        if it == 0:
                            v1 = work.tile([BLK, 160], BF16, tag="v1")
                            _squash_on(nc, work, sb, v1[:])
                        else:
                            out_t = work.tile([BLK, 160], F32, tag="out_t")
                            _squash_on(nc, work, sb, out_t[:])
                            nc.sync.dma_start(
                                out[b * BLK:(b + 1) * BLK, :], out_t[:])
    nc.compile()
    return nc


def _host_prep(inputs, W):
    """Build per-core input maps from full inputs."""
    import ml_dtypes
    x = np.ascontiguousarray(inputs, dtype=np.float32).reshape(B, R * C, IE)
    Wf = np.ascontiguousarray(W, dtype=np.float32)  # [n, i, e, o]
    # bdw[(i8_r,e), (g, f, i8, o, n2)]: delta(i8_r,i8) * W[2f+n2, 8g+i8, e, o]
    Wg = Wf.reshape(NF, 2, NCH, 8, D_IN, CAPS_DIM)  # [f, n2, g, i8, e, o]
    bdw7 = np.zeros((8, D_IN, NCH, NF, 8, CAPS_DIM, 2), dtype=np.float32)
    for i8 in range(8):
        # [f, n2, g, e, o] -> [e, g, f, o, n2]
        bdw7[i8, :, :, :, i8, :, :] = Wg[:, :, :, i8, :, :].transpose(3, 2, 0, 4, 1)
    bdw = bdw7.reshape(128, NCH * NF * 256).astype(ml_dtypes.bfloat16)
    # wdb[(i,e), (o,n)]
    wdb = np.ascontiguousarray(
        Wf.transpose(1, 2, 3, 0).reshape(IE, CAPS_DIM * N_CAPS)
    ).astype(ml_dtypes.bfloat16)
    bpc = B // N_CORES
    in_maps = []
    for c in range(N_CORES):
        xc = x[c * bpc:(c + 1) * bpc].reshape(POS, IE)
        in_maps.append({
            "xTb": np.ascontiguousarray(xc.T).astype(ml_dtypes.bfloat16),
            "bdw": bdw,
            "wdb": wdb,
        })
    return in_maps


_NC_CACHE = []


def kernel(inputs: np.ndarray, W: np.ndarray) -> np.ndarray:
    in_maps = _host_prep(inputs, W)
    if not _NC_CACHE:
        _NC_CACHE.append(build_kernel())
    nc = _NC_CACHE[0]
    res = run_bass_kernel_spmd(nc, in_maps, list(range(N_CORES)))
    outs = [res.results[c]["out"] for c in range(N_CORES)]
    full = np.concatenate(outs, axis=0)  # [3136, (o,n)]
    return np.ascontiguousarray(
        full.reshape(B, R, C, CAPS_DIM, N_CAPS).transpose(0, 1, 2, 4, 3))


# revision 25
# speedup vs baseline: 11.9890x; 2.9676x over previous
"""CapsLayer2D Trainium2 kernel (8-core SPMD, data-parallel over batch).

Math per position p (of B*R*C) and capsule n:
  U[n,i,o] = sum_e x[p,i,e] * W[n,i,e,o]          (u_hat)
  b0 = 1/64; 2x { v = squash(sum_i b*U); b += sum_o U*v }; out = squash(sum_i b*U)

Routing is algebraically restated without the b-state:
  s_mean = (1/64) sum_i U_i ; v0 = squash(s_mean); s(0) = s_mean
  iter t: a_i = U_i . v_t ; s(t+1) = s(t) + sum_i a_i U_i ; v_{t+1} = squash(s(t+1))
(identical results: b_t = 1/64 + U.(v0+..+v_{t-1}) telescopes into s).

Mapping:
  - 8 cores, 2 batches each -> 392 positions/core, 4 pos-blocks of 98.
  - Per block: s_mean via one dense bf16 matmul (K=1024, N=160); u_hat via
    block-diagonal-W bf16 matmuls (PSUM cols (gi,o,n2)), ACT-drained into
    U[p, (gi, o, n)] bf16; then 2 routing iterations on DVE.
    All routing DVE ops run in 2x mode: broadcasts sit on non-innermost dims,
    tree-sum halves stay contiguous (o middle -> o-tree 2x; gi outermost ->
    i-tree halves are contiguous monoliths).
  - Output v-layout is (o, n) per position; host transposes to (n, o).
"""
import numpy as np

import concourse.bacc as bacc
import concourse.bass as bass
import concourse.mybir as mybir
import concourse.tile as tile
from concourse.bass_utils import run_bass_kernel_spmd

N_CORES = 8
B, R, C = 16, 14, 14
N_IN, D_IN = 64, 16          # i, e
N_CAPS, CAPS_DIM = 10, 16    # n, o
IE = N_IN * D_IN             # 1024
POS = (B // N_CORES) * R * C # 392 positions per core
BLK = 98                     # pos-block size
NBLK = POS // BLK            # 4
NF = N_CAPS // 2             # 5 units of 2 capsules
NCH = IE // 128              # 8 contraction chunks
F32 = mybir.dt.float32
BF16 = mybir.dt.bfloat16
AF = mybir.ActivationFunctionType


def _squash_on(nc, pool, s_ap, v_ap):
    """v = squash(s) in (o, n) free layout. s_ap [P,160] f32, v_ap [P,160].

    Square and the Sqrt-independent steps run on DVE before the single ACT
    Sqrt dependency, minimizing the DVE stall on the ACT round-trip."""
    P = s_ap.shape[0]
    sq = pool.tile([P, 160], F32, tag="sq")
    nc.vector.tensor_mul(sq[:], s_ap, s_ap)
    q = pool.tile([P, N_CAPS], F32, tag="q")
    nc.vector.tensor_reduce(q[:], sq[:].rearrange("p (o n) -> p n o", n=N_CAPS),
                            axis=mybir.AxisListType.X, op=mybir.AluOpType.add)
    qp = pool.tile([P, N_CAPS], F32, tag="qp")
    nc.vector.tensor_scalar_add(qp[:], q[:], 1.0)
    rc = pool.tile([P, N_CAPS], F32, tag="rc")
    nc.vector.reciprocal(rc[:], qp[:])
    rt = pool.tile([P, N_CAPS], F32, tag="rt")
    nc.scalar.activation(rt[:], q[:], AF.Sqrt)
    al = pool.tile([P, N_CAPS], F32, tag="al")
    nc.vector.tensor_mul(al[:], rt[:], rc[:])
    alb = al[:].unsqueeze(1).broadcast_to([P, CAPS_DIM, N_CAPS])
    nc.vector.tensor_mul(v_ap.rearrange("p (o n) -> p o n", n=N_CAPS),
                         s_ap.rearrange("p (o n) -> p o n", n=N_CAPS), alb)


def build_kernel(dbg=False, repeat=1):
    nc = bacc.Bacc("TRN2", target_bir_lowering=False, debug=False,
                   num_devices=N_CORES)
    xTb = nc.dram_tensor("xTb", [IE, POS], BF16, kind="ExternalInput").ap()
    # bdw: [128=(i8,e), (g,f) * 256=(i8,o,n2)] block-diag W, bf16
    bdw = nc.dram_tensor("bdw", [128, NCH * NF * 256], BF16,
                         kind="ExternalInput").ap()
    # wdb: [IE, 160=(o,n)] dense W for s_mean
    wdb = nc.dram_tensor("wdb", [IE, N_CAPS * 16], BF16,
                         kind="ExternalInput").ap()
    # out rows = positions, cols = (o, n)
    out = nc.dram_tensor("out", [POS, N_CAPS * 16], F32,
                         kind="ExternalOutput").ap()

    with tile.TileContext(nc) as tc:
        for _rep in range(repeat):
            with tc.tile_pool(name="const", bufs=1) as const, \
                 tc.tile_pool(name="work", bufs=2) as work, \
                 tc.tile_pool(name="ubp", bufs=2) as ubp, \
                 tc.tile_pool(name="big", bufs=1) as big, \
                 tc.tile_pool(name="psum_u", bufs=2, space="PSUM") as psum_u:
                # Warm the ACT function tables (Copy/Sqrt) during the input
                # DMAs instead of paying the ~1.3us load on the critical path.
                warm = const.tile([1, 2], F32)
                nc.vector.memset(warm[:], 1.0)
                nc.scalar.activation(warm[:, 0:1], warm[:, 1:2], AF.Copy)
                nc.scalar.activation(warm[:, 0:1], warm[:, 1:2], AF.Sqrt)
                # xtb + wd first: they gate s_mean(0); bdw only gates u_hat(0).
                # Spread issue across engine queues so HWDGE issue pipelines.
                dmae = [nc.sync, nc.scalar, nc.sync, nc.scalar]
                xtb_t = const.tile([128, NCH * POS], BF16)
                wd_t = const.tile([128, NCH * 160], BF16)
                bdw_t = const.tile([128, NCH * NF * 256], BF16)
                for g in range(NCH):   # interleave so u_hat(0) streams early
                    nc.sync.dma_start(xtb_t[:, g * POS:(g + 1) * POS],
                                      xTb[g * 128:(g + 1) * 128, :])
                    nc.scalar.dma_start(
                        bdw_t[:, g * NF * 256:(g + 1) * NF * 256],
                        bdw[:, g * NF * 256:(g + 1) * NF * 256])
                for g in range(NCH):
                    nc.gpsimd.dma_start(wd_t[:, g * 160:(g + 1) * 160],
                                        wdb[g * 128:(g + 1) * 128, :])
                sacc = const.tile([BLK, NBLK * 160], F32)   # s per block, (o,n)
                v0_t = const.tile([BLK, NBLK * 160], BF16)

                # ---- prologue: s_mean(b) for all blocks; v0 = squash ----
                # (keeps these short ACT/DVE chains off the routing's critical
                # path -- the U-drains otherwise queue ahead of them on ACT)
                for b in range(NBLK):
                    sb = sacc[:, b * 160:(b + 1) * 160]
                    ps = psum_u.tile([BLK, 160], F32, tag="ps")
                    for g in range(NCH):
                        nc.tensor.matmul(
                            ps[:],
                            xtb_t[:, g * POS + b * BLK: g * POS + (b + 1) * BLK],
                            wd_t[:, g * 160:(g + 1) * 160],
                            start=(g == 0), stop=(g == NCH - 1))
                    nc.scalar.activation(sb, ps[:], AF.Copy, scale=1.0 / N_IN)
                    _squash_on(nc, work, sb, v0_t[:, b * 160:(b + 1) * 160])

                for b in range(NBLK):
                    sb = sacc[:, b * 160:(b + 1) * 160]
                    v0 = v0_t[:, b * 160:(b + 1) * 160]
                    # ---- u_hat(b) -> U[p, (gi, o, n)] bf16 ----
                    U = ubp.tile([BLK, 10240], BF16, tag="U")
                    Uv = U[:].rearrange("p (gi o n) -> p gi o n",
                                        gi=64, o=16, n=N_CAPS)
                    for f in range(NF):
                        for h in range(2):   # chunk halves: g in [4h, 4h+4)
                            up = psum_u.tile([BLK, 1024], F32, tag="up")
                            for gg in range(4):
                                g = 4 * h + gg
                                nc.tensor.matmul(
                                    up[:, gg * 256:(gg + 1) * 256],
                                    xtb_t[:, g * POS + b * BLK: g * POS + (b + 1) * BLK],
                                    bdw_t[:, (g * NF + f) * 256:(g * NF + f + 1) * 256],
                                    start=True, stop=True)
                            # PSUM cols (i8,o,n2) per chunk -> merged (gi,o,n2)
                            nc.scalar.activation(
                                Uv[:, 32 * h:32 * (h + 1), :, 2 * f:2 * f + 2],
                                up[:].rearrange("p (gi o n) -> p gi o n",
                                                gi=32, o=16, n=2),
                                AF.Copy)

                    # ---- 2 routing iterations ----
                    for it in range(2):
                        v_ap = v0 if it == 0 else v1[:]
                        # P = U * v (bcast over gi: middle dims stay 2x)
                        P = big.tile([BLK, 10240], BF16, tag="P")
                        Pv4 = P[:].rearrange("p (gi o n) -> p gi o n",
                                             gi=64, o=16, n=N_CAPS)
                        if b == 0 and it == 0:
                            # pipeline fill: split per (f,h)-slice so the mul
                            # starts as soon as each U-drain lands
                            for f in range(NF):
                                for h in range(2):
                                    sl = (slice(None), slice(32 * h, 32 * h + 32),
                                          slice(None), slice(2 * f, 2 * f + 2))
                                    vbs = v_ap.rearrange("p (o n) -> p o n",
                                                         n=N_CAPS) \
                                        [:, :, 2 * f:2 * f + 2].unsqueeze(1) \
                                        .broadcast_to([BLK, 32, 16, 2])
                                    nc.vector.tensor_mul(Pv4[sl], Uv[sl], vbs)
                        else:
                            vb = v_ap.rearrange("p (o n) -> p o n", n=N_CAPS) \
                                .unsqueeze(1).broadcast_to([BLK, 64, 16, N_CAPS])
                            nc.vector.tensor_mul(Pv4, Uv, vb)
                        # o-tree (middle-dim halves, contiguous runs)
                        with nc.allow_low_precision("bf16 tree sums"):
                            Pv = P[:].rearrange("p (gi o n) -> p gi o n",
                                                gi=64, o=16, n=N_CAPS)
                            t1 = big.tile([BLK, 5120], BF16, tag="t1")
                            t1v = t1[:].rearrange("p (gi o n) -> p gi o n",
                                                  gi=64, o=8, n=N_CAPS)
                            nc.vector.tensor_add(t1v, Pv[:, :, 0:8, :],
                                                 Pv[:, :, 8:16, :])
                            t2 = big.tile([BLK, 2560], BF16, tag="t2")
                            t2v = t2[:].rearrange("p (gi o n) -> p gi o n",
                                                  gi=64, o=4, n=N_CAPS)
                            nc.vector.tensor_add(t2v, t1v[:, :, 0:4, :],
                                                 t1v[:, :, 4:8, :])
                            t3 = big.tile([BLK, 1280], BF16, tag="t3")
                            t3v = t3[:].rearrange("p (gi o n) -> p gi o n",
                                                  gi=64, o=2, n=N_CAPS)
                            nc.vector.tensor_add(t3v, t2v[:, :, 0:2, :],
                                                 t2v[:, :, 2:4, :])
                            a = big.tile([BLK, 640], BF16, tag="a")
                            av = a[:].rearrange("p (gi o n) -> p gi o n",
                                                gi=64, o=1, n=N_CAPS)
                            nc.vector.tensor_add(av, t3v[:, :, 0:1, :],
                                                 t3v[:, :, 1:2, :])
                        # Q = U * a (bcast over o: middle dim, still 2x)
                        Q = big.tile([BLK, 10240], BF16, tag="Q")
                        ab = a[:].rearrange("p (gi n) -> p gi n", n=N_CAPS) \
                            .unsqueeze(2).broadcast_to([BLK, 64, 16, N_CAPS])
                        nc.vector.tensor_mul(
                            Q[:].rearrange("p (gi o n) -> p gi o n",
                                           gi=64, o=16, n=N_CAPS), Uv, ab)
                        # i-tree (outermost gi halves: contiguous monoliths)
                        with nc.allow_low_precision("bf16 tree sums"):
                            q1 = big.tile([BLK, 5120], BF16, tag="q1")
                            nc.vector.tensor_add(q1[:], Q[:, 0:5120],
                                                 Q[:, 5120:10240])
                            q2 = big.tile([BLK, 2560], BF16, tag="q2")
                            nc.vector.tensor_add(q2[:], q1[:, 0:2560],
                                                 q1[:, 2560:5120])
                            q3 = big.tile([BLK, 1280], BF16, tag="q3")
                            nc.vector.tensor_add(q3[:], q2[:, 0:1280],
                                                 q2[:, 1280:2560])
                            q4 = big.tile([BLK, 640], BF16, tag="q4")
                            nc.vector.tensor_add(q4[:], q3[:, 0:640],
                                                 q3[:, 640:1280])
                            q5 = big.tile([BLK, 320], BF16, tag="q5")
                            nc.vector.tensor_add(q5[:], q4[:, 0:320],
                                                 q4[:, 320:640])
                            inc = work.tile([BLK, 160], F32, tag="inc")
                            nc.vector.tensor_add(inc[:], q5[:, 0:160],
                                                 q5[:, 160:320])
                        nc.vector.tensor_add(sb, sb, inc[:])
                        if it == 0:
                            v1 = work.tile([BLK, 160], BF16, tag="v1")
                            _squash_on(nc, work, sb, v1[:])
                        else:
                            out_t = work.tile([BLK, 160], F32, tag="out_t")
                            _squash_on(nc, work, sb, out_t[:])
                            nc.sync.dma_start(
                                out[b * BLK:(b + 1) * BLK, :], out_t[:])
    nc.compile()
    return nc


def _host_prep(inputs, W):
    """Build per-core input maps from full inputs."""
    import ml_dtypes
    x = np.ascontiguousarray(inputs, dtype=np.float32).reshape(B, R * C, IE)
    Wf = np.ascontiguousarray(W, dtype=np.float32)  # [n, i, e, o]
    # bdw[(i8_r,e), (g, f, i8, o, n2)]: delta(i8_r,i8) * W[2f+n2, 8g+i8, e, o]
    Wg = Wf.reshape(NF, 2, NCH, 8, D_IN, CAPS_DIM)  # [f, n2, g, i8, e, o]
    bdw7 = np.zeros((8, D_IN, NCH, NF, 8, CAPS_DIM, 2), dtype=np.float32)
    for i8 in range(8):
        # [f, n2, g, e, o] -> [e, g, f, o, n2]
        bdw7[i8, :, :, :, i8, :, :] = Wg[:, :, :, i8, :, :].transpose(3, 2, 0, 4, 1)
    bdw = bdw7.reshape(128, NCH * NF * 256).astype(ml_dtypes.bfloat16)
    # wdb[(i,e), (o,n)]
    wdb = np.ascontiguousarray(
        Wf.transpose(1, 2, 3, 0).reshape(IE, CAPS_DIM * N_CAPS)
    ).astype(ml_dtypes.bfloat16)
    bpc = B // N_CORES
    in_maps = []
    for c in range(N_CORES):
        xc = x[c * bpc:(c + 1) * bpc].reshape(POS, IE)
        in_maps.append({
            "xTb": np.ascontiguousarray(xc.T).astype(ml_dtypes.bfloat16),
            "bdw": bdw,
            "wdb": wdb,
        })
    return in_maps


_NC_CACHE = []


def kernel(inputs: np.ndarray, W: np.ndarray) -> np.ndarray:
    in_maps = _host_prep(inputs, W)
    if not _NC_CACHE:
        _NC_CACHE.append(build_kernel())
    nc = _NC_CACHE[0]
    res = run_bass_kernel_spmd(nc, in_maps, list(range(N_CORES)))
    outs = [res.results[c]["out"] for c in range(N_CORES)]
    full = np.concatenate(outs, axis=0)  # [3136, (o,n)]
    return np.ascontiguousarray(
        full.reshape(B, R, C, CAPS_DIM, N_CAPS).transpose(0, 1, 2, 4, 3))
